# revision 43
# baseline (speedup 1.0000x reference)
"""Trainium2 Bass kernel for nn_Attention_40716289966507.

Reference computation (B=4, C=256, H=W=48, heads=8, d=32, N=H*W=2304):
    qkv = w_qkv @ x            # 1x1 conv -> q,k,v each [B, 256, N]
    attn = softmax(q^T k / sqrt(d))   per (batch, head): [N, N]
    out  = v @ attn^T          # [B, 256, N]
    y    = w_proj @ out + b    # [B, 256, N]

Sharding (8 cores): core i handles batch b = i//2 and query-token half
t = i%2 (1152 of the 2304 tokens). Each core needs the full image of its
batch (for K and V) but only its token half for Q; it produces the full
256-channel output for its 1152 tokens, so the host just concatenates —
no cross-core reduction.

Per-core device pipeline. The structural bottleneck is the softmax exp:
8 heads x 1152 q x 2304 k = 21.2M logits/core must each pass through a
1-elem/cycle/lane elementwise engine (GpSimd and DMA cannot touch PSUM,
so only ACT and DVE qualify). The kernel therefore splits exp between
BOTH engines, ~3:2 (EXP_DVE_PAT):
  * ACT pairs: true exp ACTIVATE (scale folded into the free affine).
  * DVE pairs: one-op Schraudolph exp - tensor_scalar(mult, add) with
    int16 output bitcast to the fp16 pt tile (see EXP_A/EXP_B). ~1.8%
    rms sawtooth on those pairs; softmax normalization cancels the
    common mode, measured end-to-end rel err ~6.6e-3 (budget 2e-2).
  * qkv/proj matmuls in bf16 (inputs pre-converted on host: halves DMA
    and avoids the f32r 4-cyc/row penalty on the 128-wide tail); q/k
    evacuated to bf16 SBUF, v materialized TRANSPOSED ([tokens, ch],
    fp16) off the tensor engine, two key-chunks per 2-bank PSUM
    super-tile -> single merged DVE evacuations.
  * Attention per 4-head group, per query tile (512/512/128-tail), per
    128-key chunk: S^T via 4 row-packed bf16 matmuls (one PSUM bank per
    head; concurrent same-bank PE drains are fatal on TRN2); exp per
    2-head pair (2 banks) routed ACT/DVE; AV + denominator matmuls
    col-packed fp16 accumulating over chunks; reciprocal_approx_fast +
    one multiply normalize AV.
  * proj in bf16 + per-channel bias via tensor_scalar, DMA out per chunk.
  * PSUM: one shared 3x2-bank rotating pool for S^T pairs AND qkv/vt/proj
    super-tiles + av + sm accumulators = exactly 8 banks; 5-deep pt ring
    in SBUF decouples the exp engines from the AV/SM consumers; v^T
    super-tiles are emitted inside the first chunk sweep (~6 chunks ahead
    of use) so their evacuations do not outrank the exp stream.
Engine budget per core (CoreSim): ACT ~103.2us, DVE ~103.7us, PE ~73us
(packing-corrected), DMA ~9us; wall ~ startup(6) + max(engines) + drain.
"""

import numpy as np

import concourse.bacc as bacc
import concourse.mybir as mybir
import concourse.tile as tile

F32 = mybir.dt.float32
F32R = mybir.dt.float32r
BF16 = mybir.dt.bfloat16
FP16 = mybir.dt.float16

P = 128
C = 256          # channels
N = 2304         # tokens per image
NQ = 1152        # query tokens per core
D = 32           # head dim
KC = N // P      # 18 key chunks
SCALE = D ** -0.5
QT = [(0, 512), (512, 512)]   # full query tiles; 1024:1152 tail is a merged pass
NT3 = 384        # free-dim tile for qkv/proj matmuls (1152 = 3*384)

# Schraudolph-style one-op exp on the DVE: for raw logits s, the fp16 bit
# pattern of exp(SCALE*s) is approximated by int16(s*EXP_A + EXP_B) (floor
# on convert), bitcast to fp16. Sawtooth rel-err ~1.8% rms; softmax
# normalization cancels the common mode, and only EXP_DVE_PAT's share of
# pairs takes this path, so the end-to-end contribution stays ~1%.
EXP_A = 261.15569832877816   # SCALE * 2^10 / ln(2)
EXP_B = 15310.0              # fp16 exponent bias<<10, shifted -50 (rms-opt)
# pair-exp engine pattern, cycled globally: True -> DVE, False -> ACT.
# (16-bit PSUM matmul output would enable DVE 2x reads, but that is a
# TRN3-only feature — on TRN2 the logits are always fp32 in PSUM. Moving
# the evacuation copies to ACT and rebalancing was tried: a wash.)
EXP_DVE_PAT = (True, False, False, True, False)


def emit(tc):
    from contextlib import ExitStack
    ctx = ExitStack()
    nc = tc.nc
    xq_d = nc.dram_tensor("xq", [C, NQ], BF16, kind="ExternalInput").ap()
    xf_d = nc.dram_tensor("xf", [C, N], BF16, kind="ExternalInput").ap()
    wqkvT_d = nc.dram_tensor("wqkvT", [C, 3 * C], BF16, kind="ExternalInput").ap()
    wprojT_d = nc.dram_tensor("wprojT", [C, C], BF16, kind="ExternalInput").ap()
    bprojT_d = nc.dram_tensor("bprojT", [P, 2], F32, kind="ExternalInput").ap()
    y_d = nc.dram_tensor("y", [C, NQ], F32, kind="ExternalOutput").ap()

    singles = ctx.enter_context(tc.tile_pool(name="singles", bufs=1))
    acts = ctx.enter_context(tc.tile_pool(name="acts", bufs=1))
    # one shared pool of 3 x 2-bank bufs serves both the S^T pair tiles and
    # the qkv/vt/proj super-tiles (phase-interleaved): the third buffer
    # gives the PE an extra chunk of runway over a dedicated 2-buf st pool.
    st_ps = ctx.enter_context(tc.tile_pool(name="st_ps", bufs=3, space="PSUM"))
    qkv_ps = st_ps
    av_ps = ctx.enter_context(tc.tile_pool(name="av_ps", bufs=1, space="PSUM"))
    sm_ps = ctx.enter_context(tc.tile_pool(name="sm_ps", bufs=1, space="PSUM"))
    # 5-deep pt ring: the exp->AV runway must absorb the pattern's
    # occasional both-ACT chunks plus vt/qkv copy bursts on the DVE
    pt_pool = ctx.enter_context(tc.tile_pool(name="pt", bufs=5))
    small = ctx.enter_context(tc.tile_pool(name="small", bufs=2))

    # preload the exp table while DMAs/qkv run
    warm = singles.tile([P, 8], F32)
    nc.vector.memset(warm[:], 0.0)
    warm2 = singles.tile([P, 8], F32)
    nc.scalar.activation(warm2[:], warm[:], mybir.ActivationFunctionType.Exp)

    ones_sb = singles.tile([P, D], FP16)
    nc.vector.memset(ones_sb[:], 1.0)
    bias_sb = singles.tile([P, 2], F32)
    nc.sync.dma_start(bias_sb[:], bprojT_d)

    # warm the PE clock gate during the input-DMA window: ~6 x 512-col
    # dummy matmuls (~2.5us cold) so the first real qkv matmuls run at
    # 2.4GHz instead of the 1.2GHz cold clock. Uses an st-pool buf that
    # is long free before the first S^T needs it.
    warm_rhs = singles.tile([P, 512], FP16)
    nc.vector.memset(warm_rhs[:], 0.0)
    warm_ps = st_ps.tile([P, 2, 512], F32, tag="st")
    for i in range(6):
        nc.tensor.matmul(warm_ps[0:D, i % 2, :], ones_sb[:, :], warm_rhs[:, :],
                         start=True, stop=True)

    # weights + x, DMA'd in the order the pipeline consumes them: q/k weight
    # sections and the first x tiles first (they gate the first S^T/exp),
    # the v weight section and remaining x tiles after.
    wq_sb = singles.tile([P, 2, 3 * C], BF16)
    wqkvT_r = wqkvT_d.rearrange("(ki p) o -> p ki o", p=P)
    for sec in range(2):          # q, k weight sections first
        for ki in range(2):
            sl = slice(sec * C, (sec + 1) * C)
            nc.sync.dma_start(wq_sb[:, ki, sl], wqkvT_r[:, ki, sl])

    xq_sb = singles.tile([P, 2, NQ], BF16)
    xq_r = xq_d.rearrange("(ki p) n -> p ki n", p=P)
    xf_sb = singles.tile([P, 2, N], BF16)
    xf_r = xf_d.rearrange("(ki p) n -> p ki n", p=P)

    def dma_xq(nt):
        sl = slice(nt * NT3, (nt + 1) * NT3)
        for ki in range(2):
            nc.sync.dma_start(xq_sb[:, ki, sl], xq_r[:, ki, sl])

    def dma_xf(nt):
        sl = slice(nt * NT3, (nt + 1) * NT3)
        for ki in range(2):
            nc.sync.dma_start(xf_sb[:, ki, sl], xf_r[:, ki, sl])

    dma_xq(0)
    dma_xq(1)
    dma_xf(0)
    for ki in range(2):           # v weight section
        nc.sync.dma_start(wq_sb[:, ki, 2 * C:3 * C], wqkvT_r[:, ki, 2 * C:3 * C])
    dma_xq(2)
    for nt in range(1, N // NT3):
        dma_xf(nt)
    wp_sb = singles.tile([P, 2, C], BF16)
    nc.sync.dma_start(wp_sb[:], wprojT_d.rearrange("(ki p) o -> p ki o", p=P))

    # per-group activations (separate tiles => fine-grained deps)
    q_g = [acts.tile([P, NQ], BF16, name=f"q{g}") for g in range(2)]
    k_g = [acts.tile([P, N], BF16, name=f"k{g}") for g in range(2)]
    vT_c2 = [acts.tile([P, 2, C], FP16, name=f"vt{j}") for j in range(KC // 2)]
    av_sb = acts.tile([P, 2, NQ], BF16)
    y_sb = acts.tile([P, 2, NQ], F32)

    def vT(kc):
        return vT_c2[kc // 2][:, kc % 2]

    mm = nc.tensor.matmul

    exp_ctr = [0]

    def emit_exp(pt_dst, st_src):
        # one softmax-exp pair instruction, routed ACT or DVE by pattern.
        # ctr 91 additionally flips ACT->DVE: one extra DVE pair balances
        # the engines (CoreSim: ACT 103.2us vs DVE 103.7us).
        use_dve = EXP_DVE_PAT[exp_ctr[0] % len(EXP_DVE_PAT)] or exp_ctr[0] == 91
        exp_ctr[0] += 1
        if use_dve:
            nc.vector.tensor_scalar(
                pt_dst.bitcast(mybir.dt.int16), st_src, EXP_A, EXP_B,
                mybir.AluOpType.mult, mybir.AluOpType.add)
        else:
            nc.scalar.activation(pt_dst, st_src,
                                 mybir.ActivationFunctionType.Exp,
                                 scale=SCALE)

    def qkv_mm(dst_tile, w_col0, rhs_sb, nts):
        # one or two NT3 sub-tiles matmul'd into a 2-bank PSUM super-tile,
        # evacuated by a single DVE copy (halves copy count + sems)
        ps = qkv_ps.tile([P, 2, 512], F32, tag="st")
        for j, nt in enumerate(nts):
            sl = slice(nt * NT3, (nt + 1) * NT3)
            for ki in range(2):
                mm(ps[:, j, :NT3], wq_sb[:, ki, w_col0:w_col0 + P],
                   rhs_sb[:, ki, sl],
                   start=(ki == 0), stop=(ki == 1))
        lo, hi = nts[0] * NT3, (nts[-1] + 1) * NT3
        dst = dst_tile[:, lo:hi].rearrange("p (j n) -> p j n", n=NT3)
        nc.vector.tensor_copy(dst, ps[:, :len(nts), :NT3])

    def emit_qkv_group(g):
        # q rows for group g = channels 128g..128g+127; k = 256+128g..
        qkv_mm(q_g[g], g * P, xq_sb, (0, 1))
        qkv_mm(q_g[g], g * P, xq_sb, (2,))
        for j in range(N // NT3 // 2):
            qkv_mm(k_g[g], C + g * P, xf_sb, (2 * j, 2 * j + 1))

    def emit_vt(j):
        # v^T for key chunks 2j, 2j+1 -> one fp16 evacuation
        ps = qkv_ps.tile([P, 2, 512], F32, tag="st")
        for i in range(2):
            mo = 2 * j + i
            for ki in range(2):
                mm(ps[:, i, :C], xf_sb[:, ki, mo * P:(mo + 1) * P],
                   wq_sb[:, ki, 2 * C:3 * C],
                   start=(ki == 0), stop=(ki == 1))
        nc.vector.tensor_copy(vT_c2[j][:], ps[:, :, :C])

    def emit_attention(g, pre_chunk=None):
        for qt_i, (q0, qtw) in enumerate(QT):
            av = av_ps.tile([P, 512], F32)
            sm = sm_ps.tile([P, 512], F32)
            for kc in range(KC):
                if pre_chunk is not None:
                    pre_chunk(qt_i, kc)
                # two 2-head pair tiles, pool bufs=2: the PE fills one
                # pair's banks while ACT still reads the other pair of
                # the previous chunk -> no exp->S^T serialization.
                pt = pt_pool.tile([P, 4, 512], FP16)
                for pair in range(2):
                    st = st_ps.tile([P, 2, 512], F32, tag="st")
                    for hh in range(2):
                        h = 2 * pair + hh
                        mm(st[:, hh, :qtw],
                           k_g[g][32 * h:32 * (h + 1), kc * P:(kc + 1) * P],
                           q_g[g][32 * h:32 * (h + 1), q0:q0 + qtw],
                           start=True, stop=True,
                           tile_position=(32 * h, 0))
                    emit_exp(pt[:, 2 * pair:2 * pair + 2, :qtw],
                             st[:, :, :qtw])
                for h in range(4):
                    mm(av[32 * h:32 * (h + 1), :qtw],
                       vT(kc)[:, 128 * g + 32 * h:128 * g + 32 * (h + 1)],
                       pt[:, h, :qtw],
                       start=(kc == 0), stop=(kc == KC - 1),
                       tile_position=(0, 32 * h), skip_group_check=True)
                for h in range(4):
                    mm(sm[32 * h:32 * (h + 1), :qtw],
                       ones_sb[:, :],
                       pt[:, h, :qtw],
                       start=(kc == 0), stop=(kc == KC - 1),
                       tile_position=(0, 32 * h), skip_group_check=True)
            rec = small.tile([P, 512], F32, tag="rec")
            nc.vector.reciprocal_approx_fast(out=rec[:, :qtw], in_=sm[:, :qtw])
            nc.vector.tensor_mul(av_sb[:, g, q0:q0 + qtw], av[:, :qtw],
                                 rec[:, :qtw])

    def emit_tail():
        # queries 1024:1152 for BOTH groups in one pass: head bank h holds
        # g0 at cols 0:128, g1 at cols 128:256. Same-row-group matmuls into
        # one bank serialize on the PE (same cells), so no concurrent
        # same-bank drains.
        q0, qtw = 1024, 128
        av = av_ps.tile([P, 512], F32)
        sm = sm_ps.tile([P, 512], F32)
        for kc in range(KC):
            pt = pt_pool.tile([P, 4, 512], FP16)
            for pair in range(2):
                st = st_ps.tile([P, 2, 512], F32, tag="st")
                for g in range(2):
                    for hh in range(2):
                        h = 2 * pair + hh
                        mm(st[:, hh, g * qtw:(g + 1) * qtw],
                           k_g[g][32 * h:32 * (h + 1), kc * P:(kc + 1) * P],
                           q_g[g][32 * h:32 * (h + 1), q0:q0 + qtw],
                           start=(g == 0), stop=(g == 1),
                           tile_position=(32 * h, 0), skip_group_check=True)
                emit_exp(pt[:, 2 * pair:2 * pair + 2, :2 * qtw],
                         st[:, :, :2 * qtw])
            for g in range(2):
                for h in range(4):
                    mm(av[32 * h:32 * (h + 1), g * qtw:(g + 1) * qtw],
                       vT(kc)[:, 128 * g + 32 * h:128 * g + 32 * (h + 1)],
                       pt[:, h, g * qtw:(g + 1) * qtw],
                       start=(kc == 0 and g == 0), stop=(kc == KC - 1 and g == 1),
                       tile_position=(0, 32 * h), skip_group_check=True)
            for g in range(2):
                for h in range(4):
                    mm(sm[32 * h:32 * (h + 1), g * qtw:(g + 1) * qtw],
                       ones_sb[:, :],
                       pt[:, h, g * qtw:(g + 1) * qtw],
                       start=(kc == 0 and g == 0), stop=(kc == KC - 1 and g == 1),
                       tile_position=(0, 32 * h), skip_group_check=True)
        rec = small.tile([P, 512], F32, tag="rec")
        nc.vector.reciprocal_approx_fast(out=rec[:, :2 * qtw], in_=sm[:, :2 * qtw])
        for g in range(2):
            nc.vector.tensor_mul(av_sb[:, g, q0:q0 + qtw],
                                 av[:, g * qtw:(g + 1) * qtw],
                                 rec[:, g * qtw:(g + 1) * qtw])

    y_r = y_d.rearrange("(co p) n -> p co n", p=P)

    def emit_proj(co, nts, add_on_act=False):
        ps = qkv_ps.tile([P, 2, 512], F32, tag="st")
        for j, nt in enumerate(nts):
            sl = slice(nt * NT3, (nt + 1) * NT3)
            for ki in range(2):
                mm(ps[:, j, :NT3], wp_sb[:, ki, co * P:(co + 1) * P],
                   av_sb[:, ki, sl],
                   start=(ki == 0), stop=(ki == 1))
        lo, hi = nts[0] * NT3, (nts[-1] + 1) * NT3
        dst = y_sb[:, co, lo:hi].rearrange("p (j n) -> p j n", n=NT3)
        if add_on_act:
            # drain path: ACT is idle after its last exp, so the final
            # chunks' bias-adds run concurrently on both engines
            nc.scalar.activation(dst, ps[:, :len(nts), :NT3],
                                 mybir.ActivationFunctionType.Identity,
                                 bias=bias_sb[:, co:co + 1])
        else:
            nc.vector.tensor_scalar_add(dst, ps[:, :len(nts), :NT3],
                                        bias_sb[:, co:co + 1])
        nc.sync.dma_start(y_r[:, co, lo:hi], y_sb[:, co, lo:hi])

    # emission order shapes Tile's priorities: group A's inputs first so
    # the first exp lands as early as possible. v^T super-tiles beyond the
    # first three are emitted INSIDE attention(0)'s first query-tile sweep,
    # a few chunks ahead of their AV consumers — emitting them all up front
    # would give their DVE evacuations priority over the (critical) exp
    # stream. proj chunks for queries 0:768 only need av_sb written by the
    # full query tiles, so they are emitted before the merged tail pass and
    # fill its engine gaps; the last proj chunks follow the tail, one on
    # the qkv PSUM pool and one on the (now idle) st pool so their matmuls
    # and bias-adds overlap instead of serializing through one pool buf.
    # Output DMA is per-chunk so results ship while later chunks compute.
    emit_qkv_group(0)
    for j in range(3):
        emit_vt(j)

    # group 1's qkv super-tiles, fed into attention(0)'s SECOND query-tile
    # sweep: emitted standalone after attention(0), their DVE evacuations
    # cannot fit the near-saturated in-phase DVE gaps and stall
    # attention(1)'s first S^T (a ~3-5us ACT bubble at the transition).
    qkv1_items = [
        lambda: qkv_mm(q_g[1], P, xq_sb, (0, 1)),
        lambda: qkv_mm(k_g[1], C + P, xf_sb, (0, 1)),
        lambda: qkv_mm(k_g[1], C + P, xf_sb, (2, 3)),
        lambda: qkv_mm(q_g[1], P, xq_sb, (2,)),
        lambda: qkv_mm(k_g[1], C + P, xf_sb, (4, 5)),
    ]

    def feeder(qt_i, kc):
        if qt_i == 0 and kc % 2 == 0:
            # first sweep: v^T tiles, ~6 chunks ahead of their AV consumers
            j = kc // 2 + 3
            if j < KC // 2:
                emit_vt(j)
        elif qt_i == 1 and kc % 3 == 1:
            i = (kc - 1) // 3
            if i < len(qkv1_items):
                qkv1_items[i]()

    emit_attention(0, pre_chunk=feeder)
    emit_attention(1)
    for co in range(2):
        emit_proj(co, (0, 1))
    emit_tail()
    emit_proj(0, (2,))
    emit_proj(1, (2,), add_on_act=True)
    ctx.close()


_NC_CACHE = None


def build_nc():
    global _NC_CACHE
    if _NC_CACHE is None:
        nc = bacc.Bacc("TRN2", target_bir_lowering=False, debug=False,
                       num_devices=8)
        with tile.TileContext(nc) as tc:
            emit(tc)
        nc.compile()
        _NC_CACHE = nc
    return _NC_CACHE


def make_in_maps(x, w_qkv, w_proj, b_proj):
    import ml_dtypes
    bf16 = ml_dtypes.bfloat16
    x = np.asarray(x, np.float32).reshape(4, C, N).astype(bf16)
    wqkvT = np.ascontiguousarray(np.asarray(w_qkv, np.float32).T).astype(bf16)
    wprojT = np.ascontiguousarray(np.asarray(w_proj, np.float32).T).astype(bf16)
    bprojT = np.ascontiguousarray(np.asarray(b_proj, np.float32).reshape(2, P).T)
    in_maps = []
    for core in range(8):
        b, t = divmod(core, 2)
        in_maps.append({
            "xq": np.ascontiguousarray(x[b][:, t * NQ:(t + 1) * NQ]),
            "xf": np.ascontiguousarray(x[b]),
            "wqkvT": wqkvT,
            "wprojT": wprojT,
            "bprojT": bprojT,
        })
    return in_maps


def assemble_output(results):
    y = np.empty((4, C, N), np.float32)
    for core in range(8):
        b, t = divmod(core, 2)
        y[b][:, t * NQ:(t + 1) * NQ] = results[core]["y"]
    return y.reshape(4, C, 48, 48)


def kernel(x, w_qkv, w_proj, b_proj):
    from concourse.bass_utils import run_bass_kernel_spmd
    nc = build_nc()
    in_maps = make_in_maps(x, w_qkv, w_proj, b_proj)
    res = run_bass_kernel_spmd(nc, in_maps, core_ids=list(range(8)))
    return assemble_output(res.results)



# revision 51
# speedup vs baseline: 1.7253x; 1.7253x over previous
"""Trainium2 Bass kernel for nn_Attention_40716289966507.

Reference computation (B=4, C=256, H=W=48, heads=8, d=32, N=H*W=2304):
    qkv = w_qkv @ x            # 1x1 conv -> q,k,v each [B, 256, N]
    attn = softmax(q^T k / sqrt(d))   per (batch, head): [N, N]
    out  = v @ attn^T          # [B, 256, N]
    y    = w_proj @ out + b    # [B, 256, N]

Sharding (8 cores): core i handles batch b = i//2 and query-token half
t = i%2 (1152 of the 2304 tokens). Each core needs the full image of its
batch (for K and V) but only its token half for Q; it produces the full
256-channel output for its 1152 tokens, so the host just concatenates —
no cross-core reduction.

Per-core device pipeline. The structural bottleneck is the softmax exp:
8 heads x 1152 q x 2304 k = 21.2M logits/core must each pass through a
1-elem/cycle/lane elementwise engine (GpSimd and DMA cannot touch PSUM,
so only ACT and DVE qualify). The kernel therefore splits exp between
BOTH engines, ~3:2 (EXP_DVE_PAT):
  * ACT pairs: true exp ACTIVATE (scale folded into the free affine).
  * DVE pairs: one-op Schraudolph exp - tensor_scalar(mult, add) with
    int16 output bitcast to the fp16 pt tile (see EXP_A/EXP_B). ~1.8%
    rms sawtooth on those pairs; softmax normalization cancels the
    common mode, measured end-to-end rel err ~6.6e-3 (budget 2e-2).
  * qkv/proj matmuls in bf16 (inputs pre-converted on host: halves DMA
    and avoids the f32r 4-cyc/row penalty on the 128-wide tail); q/k
    evacuated to bf16 SBUF, v materialized TRANSPOSED ([tokens, ch],
    fp16) off the tensor engine, two key-chunks per 2-bank PSUM
    super-tile -> single merged DVE evacuations.
  * Attention per 4-head group, per query tile (512/512/128-tail), per
    128-key chunk: S^T via 4 row-packed bf16 matmuls (one PSUM bank per
    head; concurrent same-bank PE drains are fatal on TRN2); exp per
    2-head pair (2 banks) routed ACT/DVE; AV + denominator matmuls
    col-packed fp16 accumulating over chunks; reciprocal_approx_fast +
    one multiply normalize AV.
  * proj in bf16 + per-channel bias via tensor_scalar, DMA out per chunk.
  * PSUM: one shared 3x2-bank rotating pool for S^T pairs AND qkv/vt/proj
    super-tiles + av + sm accumulators = exactly 8 banks; 5-deep pt ring
    in SBUF decouples the exp engines from the AV/SM consumers; v^T
    super-tiles are emitted inside the first chunk sweep (~6 chunks ahead
    of use) and group 1's qkv super-tiles inside the second, so their
    evacuations neither outrank the exp stream nor pile up at the
    attention(0)->(1) transition; the final proj bias-add runs on the
    (drain-idle) ACT so the last two chunks evacuate concurrently.
Engine budget per core (CoreSim): ACT ~103.2us, DVE ~103.7us, PE ~73us
(packing-corrected), DMA ~9us; wall ~ startup(6) + max(engines) + drain.
"""

import numpy as np

import concourse.bacc as bacc
import concourse.mybir as mybir
import concourse.tile as tile

F32 = mybir.dt.float32
F32R = mybir.dt.float32r
BF16 = mybir.dt.bfloat16
FP16 = mybir.dt.float16

P = 128
C = 256          # channels
N = 2304         # tokens per image
NQ = 1152        # query tokens per core
D = 32           # head dim
KC = N // P      # 18 key chunks
SCALE = D ** -0.5
QT = [(0, 512), (512, 512)]   # full query tiles; 1024:1152 tail is a merged pass
NT3 = 384        # free-dim tile for qkv/proj matmuls (1152 = 3*384)

# Schraudolph-style one-op exp on the DVE: for raw logits s, the fp16 bit
# pattern of exp(SCALE*s) is approximated by int16(s*EXP_A + EXP_B) (floor
# on convert), bitcast to fp16. Sawtooth rel-err ~1.8% rms; softmax
# normalization cancels the common mode, and only EXP_DVE_PAT's share of
# pairs takes this path, so the end-to-end contribution stays ~1%.
EXP_A = 261.15569832877816   # SCALE * 2^10 / ln(2)
EXP_B = 15310.0              # fp16 exponent bias<<10, shifted -50 (rms-opt)
# pair-exp engine pattern, cycled globally: True -> DVE, False -> ACT.
# (16-bit PSUM matmul output would enable DVE 2x reads, but that is a
# TRN3-only feature — on TRN2 the logits are always fp32 in PSUM. Moving
# the evacuation copies to ACT and rebalancing was tried: a wash.)
EXP_DVE_PAT = (True, False, False, True, False)


def emit(tc):
    from contextlib import ExitStack
    ctx = ExitStack()
    nc = tc.nc
    xq_d = nc.dram_tensor("xq", [C, NQ], BF16, kind="ExternalInput").ap()
    xf_d = nc.dram_tensor("xf", [C, N], BF16, kind="ExternalInput").ap()
    wqkvT_d = nc.dram_tensor("wqkvT", [C, 3 * C], BF16, kind="ExternalInput").ap()
    wprojT_d = nc.dram_tensor("wprojT", [C, C], BF16, kind="ExternalInput").ap()
    bprojT_d = nc.dram_tensor("bprojT", [P, 2], F32, kind="ExternalInput").ap()
    y_d = nc.dram_tensor("y", [C, NQ], F32, kind="ExternalOutput").ap()

    singles = ctx.enter_context(tc.tile_pool(name="singles", bufs=1))
    acts = ctx.enter_context(tc.tile_pool(name="acts", bufs=1))
    # one shared pool of 3 x 2-bank bufs serves both the S^T pair tiles and
    # the qkv/vt/proj super-tiles (phase-interleaved): the third buffer
    # gives the PE an extra chunk of runway over a dedicated 2-buf st pool.
    st_ps = ctx.enter_context(tc.tile_pool(name="st_ps", bufs=3, space="PSUM"))
    qkv_ps = st_ps
    av_ps = ctx.enter_context(tc.tile_pool(name="av_ps", bufs=1, space="PSUM"))
    sm_ps = ctx.enter_context(tc.tile_pool(name="sm_ps", bufs=1, space="PSUM"))
    # 5-deep pt ring: the exp->AV runway must absorb the pattern's
    # occasional both-ACT chunks plus vt/qkv copy bursts on the DVE
    pt_pool = ctx.enter_context(tc.tile_pool(name="pt", bufs=5))
    small = ctx.enter_context(tc.tile_pool(name="small", bufs=2))

    # preload the exp table while DMAs/qkv run
    warm = singles.tile([P, 8], F32)
    nc.vector.memset(warm[:], 0.0)
    warm2 = singles.tile([P, 8], F32)
    nc.scalar.activation(warm2[:], warm[:], mybir.ActivationFunctionType.Exp)

    ones_sb = singles.tile([P, D], FP16)
    nc.vector.memset(ones_sb[:], 1.0)
    bias_sb = singles.tile([P, 2], F32)
    nc.sync.dma_start(bias_sb[:], bprojT_d)

    # warm the PE clock gate during the input-DMA window: ~6 x 512-col
    # dummy matmuls (~2.5us cold) so the first real qkv matmuls run at
    # 2.4GHz instead of the 1.2GHz cold clock. Uses an st-pool buf that
    # is long free before the first S^T needs it.
    warm_rhs = singles.tile([P, 512], FP16)
    nc.vector.memset(warm_rhs[:], 0.0)
    warm_ps = st_ps.tile([P, 2, 512], F32, tag="st")
    for i in range(6):
        nc.tensor.matmul(warm_ps[0:D, i % 2, :], ones_sb[:, :], warm_rhs[:, :],
                         start=True, stop=True)

    # weights + x, DMA'd in the order the pipeline consumes them: q/k weight
    # sections and the first x tiles first (they gate the first S^T/exp),
    # the v weight section and remaining x tiles after.
    wq_sb = singles.tile([P, 2, 3 * C], BF16)
    wqkvT_r = wqkvT_d.rearrange("(ki p) o -> p ki o", p=P)
    for sec in range(2):          # q, k weight sections first
        for ki in range(2):
            sl = slice(sec * C, (sec + 1) * C)
            nc.sync.dma_start(wq_sb[:, ki, sl], wqkvT_r[:, ki, sl])

    xq_sb = singles.tile([P, 2, NQ], BF16)
    xq_r = xq_d.rearrange("(ki p) n -> p ki n", p=P)
    xf_sb = singles.tile([P, 2, N], BF16)
    xf_r = xf_d.rearrange("(ki p) n -> p ki n", p=P)

    def dma_xq(nt):
        sl = slice(nt * NT3, (nt + 1) * NT3)
        for ki in range(2):
            nc.sync.dma_start(xq_sb[:, ki, sl], xq_r[:, ki, sl])

    def dma_xf(nt):
        sl = slice(nt * NT3, (nt + 1) * NT3)
        for ki in range(2):
            nc.sync.dma_start(xf_sb[:, ki, sl], xf_r[:, ki, sl])

    dma_xq(0)
    dma_xq(1)
    dma_xf(0)
    for ki in range(2):           # v weight section
        nc.sync.dma_start(wq_sb[:, ki, 2 * C:3 * C], wqkvT_r[:, ki, 2 * C:3 * C])
    dma_xq(2)
    for nt in range(1, N // NT3):
        dma_xf(nt)
    wp_sb = singles.tile([P, 2, C], BF16)
    nc.sync.dma_start(wp_sb[:], wprojT_d.rearrange("(ki p) o -> p ki o", p=P))

    # per-group activations (separate tiles => fine-grained deps)
    q_g = [acts.tile([P, NQ], BF16, name=f"q{g}") for g in range(2)]
    k_g = [acts.tile([P, N], BF16, name=f"k{g}") for g in range(2)]
    vT_c2 = [acts.tile([P, 2, C], FP16, name=f"vt{j}") for j in range(KC // 2)]
    av_sb = acts.tile([P, 2, NQ], BF16)
    y_sb = acts.tile([P, 2, NQ], F32)

    def vT(kc):
        return vT_c2[kc // 2][:, kc % 2]

    mm = nc.tensor.matmul

    exp_ctr = [0]

    def emit_exp(pt_dst, st_src):
        # one softmax-exp pair instruction, routed ACT or DVE by pattern.
        # ctr 91 additionally flips ACT->DVE: one extra DVE pair balances
        # the engines (CoreSim: ACT 103.2us vs DVE 103.7us).
        use_dve = EXP_DVE_PAT[exp_ctr[0] % len(EXP_DVE_PAT)] or exp_ctr[0] == 91
        exp_ctr[0] += 1
        if use_dve:
            nc.vector.tensor_scalar(
                pt_dst.bitcast(mybir.dt.int16), st_src, EXP_A, EXP_B,
                mybir.AluOpType.mult, mybir.AluOpType.add)
        else:
            nc.scalar.activation(pt_dst, st_src,
                                 mybir.ActivationFunctionType.Exp,
                                 scale=SCALE)

    def qkv_mm(dst_tile, w_col0, rhs_sb, nts):
        # one or two NT3 sub-tiles matmul'd into a 2-bank PSUM super-tile,
        # evacuated by a single DVE copy (halves copy count + sems)
        ps = qkv_ps.tile([P, 2, 512], F32, tag="st")
        for j, nt in enumerate(nts):
            sl = slice(nt * NT3, (nt + 1) * NT3)
            for ki in range(2):
                mm(ps[:, j, :NT3], wq_sb[:, ki, w_col0:w_col0 + P],
                   rhs_sb[:, ki, sl],
                   start=(ki == 0), stop=(ki == 1))
        lo, hi = nts[0] * NT3, (nts[-1] + 1) * NT3
        dst = dst_tile[:, lo:hi].rearrange("p (j n) -> p j n", n=NT3)
        nc.vector.tensor_copy(dst, ps[:, :len(nts), :NT3])

    def emit_qkv_group(g):
        # q rows for group g = channels 128g..128g+127; k = 256+128g..
        qkv_mm(q_g[g], g * P, xq_sb, (0, 1))
        qkv_mm(q_g[g], g * P, xq_sb, (2,))
        for j in range(N // NT3 // 2):
            qkv_mm(k_g[g], C + g * P, xf_sb, (2 * j, 2 * j + 1))

    def emit_vt(j):
        # v^T for key chunks 2j, 2j+1 -> one fp16 evacuation
        ps = qkv_ps.tile([P, 2, 512], F32, tag="st")
        for i in range(2):
            mo = 2 * j + i
            for ki in range(2):
                mm(ps[:, i, :C], xf_sb[:, ki, mo * P:(mo + 1) * P],
                   wq_sb[:, ki, 2 * C:3 * C],
                   start=(ki == 0), stop=(ki == 1))
        nc.vector.tensor_copy(vT_c2[j][:], ps[:, :, :C])

    def emit_attention(g, pre_chunk=None):
        for qt_i, (q0, qtw) in enumerate(QT):
            av = av_ps.tile([P, 512], F32)
            sm = sm_ps.tile([P, 512], F32)
            for kc in range(KC):
                if pre_chunk is not None:
                    pre_chunk(qt_i, kc)
                # two 2-head pair tiles, pool bufs=2: the PE fills one
                # pair's banks while ACT still reads the other pair of
                # the previous chunk -> no exp->S^T serialization.
                pt = pt_pool.tile([P, 4, 512], FP16)
                for pair in range(2):
                    st = st_ps.tile([P, 2, 512], F32, tag="st")
                    for hh in range(2):
                        h = 2 * pair + hh
                        mm(st[:, hh, :qtw],
                           k_g[g][32 * h:32 * (h + 1), kc * P:(kc + 1) * P],
                           q_g[g][32 * h:32 * (h + 1), q0:q0 + qtw],
                           start=True, stop=True,
                           tile_position=(32 * h, 0))
                    emit_exp(pt[:, 2 * pair:2 * pair + 2, :qtw],
                             st[:, :, :qtw])
                for h in range(4):
                    mm(av[32 * h:32 * (h + 1), :qtw],
                       vT(kc)[:, 128 * g + 32 * h:128 * g + 32 * (h + 1)],
                       pt[:, h, :qtw],
                       start=(kc == 0), stop=(kc == KC - 1),
                       tile_position=(0, 32 * h), skip_group_check=True)
                for h in range(4):
                    mm(sm[32 * h:32 * (h + 1), :qtw],
                       ones_sb[:, :],
                       pt[:, h, :qtw],
                       start=(kc == 0), stop=(kc == KC - 1),
                       tile_position=(0, 32 * h), skip_group_check=True)
            rec = small.tile([P, 512], F32, tag="rec")
            nc.vector.reciprocal_approx_fast(out=rec[:, :qtw], in_=sm[:, :qtw])
            nc.vector.tensor_mul(av_sb[:, g, q0:q0 + qtw], av[:, :qtw],
                                 rec[:, :qtw])

    def emit_tail():
        # queries 1024:1152 for BOTH groups in one pass: head bank h holds
        # g0 at cols 0:128, g1 at cols 128:256. Same-row-group matmuls into
        # one bank serialize on the PE (same cells), so no concurrent
        # same-bank drains.
        q0, qtw = 1024, 128
        av = av_ps.tile([P, 512], F32)
        sm = sm_ps.tile([P, 512], F32)
        for kc in range(KC):
            pt = pt_pool.tile([P, 4, 512], FP16)
            for pair in range(2):
                st = st_ps.tile([P, 2, 512], F32, tag="st")
                for g in range(2):
                    for hh in range(2):
                        h = 2 * pair + hh
                        mm(st[:, hh, g * qtw:(g + 1) * qtw],
                           k_g[g][32 * h:32 * (h + 1), kc * P:(kc + 1) * P],
                           q_g[g][32 * h:32 * (h + 1), q0:q0 + qtw],
                           start=(g == 0), stop=(g == 1),
                           tile_position=(32 * h, 0), skip_group_check=True)
                emit_exp(pt[:, 2 * pair:2 * pair + 2, :2 * qtw],
                         st[:, :, :2 * qtw])
            for g in range(2):
                for h in range(4):
                    mm(av[32 * h:32 * (h + 1), g * qtw:(g + 1) * qtw],
                       vT(kc)[:, 128 * g + 32 * h:128 * g + 32 * (h + 1)],
                       pt[:, h, g * qtw:(g + 1) * qtw],
                       start=(kc == 0 and g == 0), stop=(kc == KC - 1 and g == 1),
                       tile_position=(0, 32 * h), skip_group_check=True)
            for g in range(2):
                for h in range(4):
                    mm(sm[32 * h:32 * (h + 1), g * qtw:(g + 1) * qtw],
                       ones_sb[:, :],
                       pt[:, h, g * qtw:(g + 1) * qtw],
                       start=(kc == 0 and g == 0), stop=(kc == KC - 1 and g == 1),
                       tile_position=(0, 32 * h), skip_group_check=True)
        rec = small.tile([P, 512], F32, tag="rec")
        nc.vector.reciprocal_approx_fast(out=rec[:, :2 * qtw], in_=sm[:, :2 * qtw])
        for g in range(2):
            nc.vector.tensor_mul(av_sb[:, g, q0:q0 + qtw],
                                 av[:, g * qtw:(g + 1) * qtw],
                                 rec[:, g * qtw:(g + 1) * qtw])

    y_r = y_d.rearrange("(co p) n -> p co n", p=P)

    def emit_proj(co, nts, add_on_act=False):
        ps = qkv_ps.tile([P, 2, 512], F32, tag="st")
        for j, nt in enumerate(nts):
            sl = slice(nt * NT3, (nt + 1) * NT3)
            for ki in range(2):
                mm(ps[:, j, :NT3], wp_sb[:, ki, co * P:(co + 1) * P],
                   av_sb[:, ki, sl],
                   start=(ki == 0), stop=(ki == 1))
        lo, hi = nts[0] * NT3, (nts[-1] + 1) * NT3
        dst = y_sb[:, co, lo:hi].rearrange("p (j n) -> p j n", n=NT3)
        if add_on_act:
            # drain path: ACT is idle after its last exp, so the final
            # chunks' bias-adds run concurrently on both engines
            nc.scalar.activation(dst, ps[:, :len(nts), :NT3],
                                 mybir.ActivationFunctionType.Identity,
                                 bias=bias_sb[:, co:co + 1])
        else:
            nc.vector.tensor_scalar_add(dst, ps[:, :len(nts), :NT3],
                                        bias_sb[:, co:co + 1])
        nc.sync.dma_start(y_r[:, co, lo:hi], y_sb[:, co, lo:hi])

    # emission order shapes Tile's priorities: group A's inputs first so
    # the first exp lands as early as possible. v^T super-tiles beyond the
    # first three are emitted INSIDE attention(0)'s first query-tile sweep,
    # a few chunks ahead of their AV consumers — emitting them all up front
    # would give their DVE evacuations priority over the (critical) exp
    # stream. proj chunks for queries 0:768 only need av_sb written by the
    # full query tiles, so they are emitted before the merged tail pass and
    # fill its engine gaps; the last proj chunks follow the tail, one on
    # the qkv PSUM pool and one on the (now idle) st pool so their matmuls
    # and bias-adds overlap instead of serializing through one pool buf.
    # Output DMA is per-chunk so results ship while later chunks compute.
    emit_qkv_group(0)
    for j in range(3):
        emit_vt(j)

    # group 1's qkv super-tiles, fed into attention(0)'s SECOND query-tile
    # sweep: emitted standalone after attention(0), their DVE evacuations
    # cannot fit the near-saturated in-phase DVE gaps and stall
    # attention(1)'s first S^T (a ~3-5us ACT bubble at the transition).
    qkv1_items = [
        lambda: qkv_mm(q_g[1], P, xq_sb, (0, 1)),
        lambda: qkv_mm(k_g[1], C + P, xf_sb, (0, 1)),
        lambda: qkv_mm(k_g[1], C + P, xf_sb, (2, 3)),
        lambda: qkv_mm(q_g[1], P, xq_sb, (2,)),
        lambda: qkv_mm(k_g[1], C + P, xf_sb, (4, 5)),
    ]

    def feeder(qt_i, kc):
        if qt_i == 0 and kc % 2 == 0:
            # first sweep: v^T tiles, ~6 chunks ahead of their AV consumers
            j = kc // 2 + 3
            if j < KC // 2:
                emit_vt(j)
        elif qt_i == 1 and kc % 3 == 1:
            i = (kc - 1) // 3
            if i < len(qkv1_items):
                qkv1_items[i]()

    emit_attention(0, pre_chunk=feeder)
    emit_attention(1)
    for co in range(2):
        emit_proj(co, (0, 1))
    emit_tail()
    emit_proj(0, (2,))
    emit_proj(1, (2,), add_on_act=True)
    ctx.close()


_NC_CACHE = None


def build_nc():
    global _NC_CACHE
    if _NC_CACHE is None:
        nc = bacc.Bacc("TRN2", target_bir_lowering=False, debug=False,
                       num_devices=8)
        with tile.TileContext(nc) as tc:
            emit(tc)
        nc.compile()
        _NC_CACHE = nc
    return _NC_CACHE


def make_in_maps(x, w_qkv, w_proj, b_proj):
    import ml_dtypes
    bf16 = ml_dtypes.bfloat16
    x = np.asarray(x, np.float32).reshape(4, C, N).astype(bf16)
    wqkvT = np.ascontiguousarray(np.asarray(w_qkv, np.float32).T).astype(bf16)
    wprojT = np.ascontiguousarray(np.asarray(w_proj, np.float32).T).astype(bf16)
    bprojT = np.ascontiguousarray(np.asarray(b_proj, np.float32).reshape(2, P).T)
    in_maps = []
    for core in range(8):
        b, t = divmod(core, 2)
        in_maps.append({
            "xq": np.ascontiguousarray(x[b][:, t * NQ:(t + 1) * NQ]),
            "xf": np.ascontiguousarray(x[b]),
            "wqkvT": wqkvT,
            "wprojT": wprojT,
            "bprojT": bprojT,
        })
    return in_maps


def assemble_output(results):
    y = np.empty((4, C, N), np.float32)
    for core in range(8):
        b, t = divmod(core, 2)
        y[b][:, t * NQ:(t + 1) * NQ] = results[core]["y"]
    return y.reshape(4, C, 48, 48)


def kernel(x, w_qkv, w_proj, b_proj):
    from concourse.bass_utils import run_bass_kernel_spmd
    nc = build_nc()
    in_maps = make_in_maps(x, w_qkv, w_proj, b_proj)
    res = run_bass_kernel_spmd(nc, in_maps, core_ids=list(range(8)))
    return assemble_output(res.results)



# revision 52
# speedup vs baseline: 4.6915x; 2.7193x over previous
"""Trainium2 Bass kernel for nn_Attention_40716289966507.

Reference computation (B=4, C=256, H=W=48, heads=8, d=32, N=H*W=2304):
    qkv = w_qkv @ x            # 1x1 conv -> q,k,v each [B, 256, N]
    attn = softmax(q^T k / sqrt(d))   per (batch, head): [N, N]
    out  = v @ attn^T          # [B, 256, N]
    y    = w_proj @ out + b    # [B, 256, N]

Sharding (8 cores): core i handles batch b = i//2 and query-token half
t = i%2 (1152 of the 2304 tokens). Each core needs the full image of its
batch (for K and V) but only its token half for Q; it produces the full
256-channel output for its 1152 tokens, so the host just concatenates —
no cross-core reduction.

Per-core device pipeline. The structural bottleneck is the softmax exp:
8 heads x 1152 q x 2304 k = 21.2M logits/core must each pass through a
1-elem/cycle/lane elementwise engine (GpSimd and DMA cannot touch PSUM,
so only ACT and DVE qualify). The kernel therefore splits exp between
BOTH engines, ~3:2 (EXP_DVE_PAT):
  * ACT pairs: true exp ACTIVATE (scale folded into the free affine).
  * DVE pairs: one-op Schraudolph exp - tensor_scalar(mult, add) with
    int16 output bitcast to the fp16 pt tile (see EXP_A/EXP_B). ~1.8%
    rms sawtooth on those pairs; softmax normalization cancels the
    common mode, measured end-to-end rel err ~6.6e-3 (budget 2e-2).
  * qkv/proj matmuls in bf16 (inputs pre-converted on host: halves DMA
    and avoids the f32r 4-cyc/row penalty on the 128-wide tail); q/k
    evacuated to bf16 SBUF, v materialized TRANSPOSED ([tokens, ch],
    fp16) off the tensor engine, two key-chunks per 2-bank PSUM
    super-tile -> single merged DVE evacuations.
  * Attention per 4-head group, per query tile (512/512/128-tail), per
    128-key chunk: S^T via 4 row-packed bf16 matmuls (one PSUM bank per
    head; concurrent same-bank PE drains are fatal on TRN2); exp per
    2-head pair (2 banks) routed ACT/DVE; AV + denominator matmuls
    col-packed fp16 accumulating over chunks; reciprocal_approx_fast +
    one multiply normalize AV.
  * proj in bf16 + per-channel bias via tensor_scalar, DMA out per chunk.
  * PSUM: one shared 3x2-bank rotating pool for S^T pairs AND qkv/vt/proj
    super-tiles + av + sm accumulators = exactly 8 banks; 5-deep pt ring
    in SBUF decouples the exp engines from the AV/SM consumers; v^T
    super-tiles are emitted inside the first chunk sweep (~6 chunks ahead
    of use) and group 1's qkv super-tiles inside the second, so their
    evacuations neither outrank the exp stream nor pile up at the
    attention(0)->(1) transition; the final proj bias-add runs on the
    (drain-idle) ACT so the last two chunks evacuate concurrently.
Engine budget per core (CoreSim): ACT ~103.2us, DVE ~103.7us, PE ~73us
(packing-corrected), DMA ~9us; wall ~ startup(6) + max(engines) + drain.
"""

import numpy as np

import concourse.bacc as bacc
import concourse.mybir as mybir
import concourse.tile as tile

F32 = mybir.dt.float32
F32R = mybir.dt.float32r
BF16 = mybir.dt.bfloat16
FP16 = mybir.dt.float16

P = 128
C = 256          # channels
N = 2304         # tokens per image
NQ = 1152        # query tokens per core
D = 32           # head dim
KC = N // P      # 18 key chunks
SCALE = D ** -0.5
QT = [(0, 512), (512, 512)]   # full query tiles; 1024:1152 tail is a merged pass
NT3 = 384        # free-dim tile for qkv/proj matmuls (1152 = 3*384)

# Schraudolph-style one-op exp on the DVE: for raw logits s, the fp16 bit
# pattern of exp(SCALE*s) is approximated by int16(s*EXP_A + EXP_B) (floor
# on convert), bitcast to fp16. Sawtooth rel-err ~1.8% rms; softmax
# normalization cancels the common mode, and only EXP_DVE_PAT's share of
# pairs takes this path, so the end-to-end contribution stays ~1%.
EXP_A = 261.15569832877816   # SCALE * 2^10 / ln(2)
EXP_B = 15310.0              # fp16 exponent bias<<10, shifted -50 (rms-opt)
# pair-exp engine pattern, cycled globally: True -> DVE, False -> ACT.
# (16-bit PSUM matmul output would enable DVE 2x reads, but that is a
# TRN3-only feature — on TRN2 the logits are always fp32 in PSUM. Moving
# the evacuation copies to ACT and rebalancing was tried: a wash.)
EXP_DVE_PAT = (True, False, False, True, False)


def emit(tc):
    from contextlib import ExitStack
    ctx = ExitStack()
    nc = tc.nc
    xq_d = nc.dram_tensor("xq", [C, NQ], BF16, kind="ExternalInput").ap()
    xf_d = nc.dram_tensor("xf", [C, N], BF16, kind="ExternalInput").ap()
    wqkvT_d = nc.dram_tensor("wqkvT", [C, 3 * C], BF16, kind="ExternalInput").ap()
    wprojT_d = nc.dram_tensor("wprojT", [C, C], BF16, kind="ExternalInput").ap()
    bprojT_d = nc.dram_tensor("bprojT", [P, 2], F32, kind="ExternalInput").ap()
    y_d = nc.dram_tensor("y", [C, NQ], F32, kind="ExternalOutput").ap()

    singles = ctx.enter_context(tc.tile_pool(name="singles", bufs=1))
    acts = ctx.enter_context(tc.tile_pool(name="acts", bufs=1))
    # one shared pool of 3 x 2-bank bufs serves both the S^T pair tiles and
    # the qkv/vt/proj super-tiles (phase-interleaved): the third buffer
    # gives the PE an extra chunk of runway over a dedicated 2-buf st pool.
    st_ps = ctx.enter_context(tc.tile_pool(name="st_ps", bufs=3, space="PSUM"))
    qkv_ps = st_ps
    av_ps = ctx.enter_context(tc.tile_pool(name="av_ps", bufs=1, space="PSUM"))
    sm_ps = ctx.enter_context(tc.tile_pool(name="sm_ps", bufs=1, space="PSUM"))
    # 5-deep pt ring: the exp->AV runway must absorb the pattern's
    # occasional both-ACT chunks plus vt/qkv copy bursts on the DVE
    pt_pool = ctx.enter_context(tc.tile_pool(name="pt", bufs=5))
    small = ctx.enter_context(tc.tile_pool(name="small", bufs=2))

    # preload the exp table while DMAs/qkv run
    warm = singles.tile([P, 8], F32)
    nc.vector.memset(warm[:], 0.0)
    warm2 = singles.tile([P, 8], F32)
    nc.scalar.activation(warm2[:], warm[:], mybir.ActivationFunctionType.Exp)

    ones_sb = singles.tile([P, D], FP16)
    nc.vector.memset(ones_sb[:], 1.0)
    bias_sb = singles.tile([P, 2], F32)
    nc.sync.dma_start(bias_sb[:], bprojT_d)

    # warm the PE clock gate during the input-DMA window: ~6 x 512-col
    # dummy matmuls (~2.5us cold) so the first real qkv matmuls run at
    # 2.4GHz instead of the 1.2GHz cold clock. Uses an st-pool buf that
    # is long free before the first S^T needs it.
    warm_rhs = singles.tile([P, 512], FP16)
    nc.vector.memset(warm_rhs[:], 0.0)
    warm_ps = st_ps.tile([P, 2, 512], F32, tag="st")
    for i in range(6):
        nc.tensor.matmul(warm_ps[0:D, i % 2, :], ones_sb[:, :], warm_rhs[:, :],
                         start=True, stop=True)

    # weights + x, DMA'd in the order the pipeline consumes them: q/k weight
    # sections and the first x tiles first (they gate the first S^T/exp),
    # the v weight section and remaining x tiles after.
    wq_sb = singles.tile([P, 2, 3 * C], BF16)
    wqkvT_r = wqkvT_d.rearrange("(ki p) o -> p ki o", p=P)
    for sec in range(2):          # q, k weight sections first
        for ki in range(2):
            sl = slice(sec * C, (sec + 1) * C)
            nc.sync.dma_start(wq_sb[:, ki, sl], wqkvT_r[:, ki, sl])

    xq_sb = singles.tile([P, 2, NQ], BF16)
    xq_r = xq_d.rearrange("(ki p) n -> p ki n", p=P)
    xf_sb = singles.tile([P, 2, N], BF16)
    xf_r = xf_d.rearrange("(ki p) n -> p ki n", p=P)

    def dma_xq(nt):
        sl = slice(nt * NT3, (nt + 1) * NT3)
        for ki in range(2):
            nc.sync.dma_start(xq_sb[:, ki, sl], xq_r[:, ki, sl])

    def dma_xf(nt):
        sl = slice(nt * NT3, (nt + 1) * NT3)
        for ki in range(2):
            nc.sync.dma_start(xf_sb[:, ki, sl], xf_r[:, ki, sl])

    dma_xq(0)
    dma_xq(1)
    dma_xf(0)
    for ki in range(2):           # v weight section
        nc.sync.dma_start(wq_sb[:, ki, 2 * C:3 * C], wqkvT_r[:, ki, 2 * C:3 * C])
    dma_xq(2)
    for nt in range(1, N // NT3):
        dma_xf(nt)
    wp_sb = singles.tile([P, 2, C], BF16)
    nc.sync.dma_start(wp_sb[:], wprojT_d.rearrange("(ki p) o -> p ki o", p=P))

    # per-group activations (separate tiles => fine-grained deps)
    q_g = [acts.tile([P, NQ], BF16, name=f"q{g}") for g in range(2)]
    k_g = [acts.tile([P, N], BF16, name=f"k{g}") for g in range(2)]
    vT_c2 = [acts.tile([P, 2, C], FP16, name=f"vt{j}") for j in range(KC // 2)]
    av_sb = acts.tile([P, 2, NQ], BF16)
    y_sb = acts.tile([P, 2, NQ], F32)

    def vT(kc):
        return vT_c2[kc // 2][:, kc % 2]

    mm = nc.tensor.matmul

    # greedy phase-local engine balancer: each exp pair goes to whichever
    # engine has less cumulative modeled work; DVE-only work (copies,
    # norm, recip) is charged as it is emitted, so exps route to ACT
    # exactly during the DVE's evacuation bursts (the static pattern was
    # globally balanced but phase-blind).
    eng_ns = [0.0, 0.0]   # [DVE, ACT]

    def dve_work(nelem):
        eng_ns[0] += (nelem + 120) / 0.96

    def emit_exp(pt_dst, st_src, nelem):
        dve_cost = (nelem + 120) / 0.96
        act_cost = (nelem + 222) / 1.2
        if eng_ns[0] + dve_cost <= eng_ns[1] + act_cost:
            eng_ns[0] += dve_cost
            nc.vector.tensor_scalar(
                pt_dst.bitcast(mybir.dt.int16), st_src, EXP_A, EXP_B,
                mybir.AluOpType.mult, mybir.AluOpType.add)
        else:
            eng_ns[1] += act_cost
            nc.scalar.activation(pt_dst, st_src,
                                 mybir.ActivationFunctionType.Exp,
                                 scale=SCALE)

    def qkv_mm(dst_tile, w_col0, rhs_sb, nts):
        # one or two NT3 sub-tiles matmul'd into a 2-bank PSUM super-tile,
        # evacuated by a single DVE copy (halves copy count + sems)
        ps = qkv_ps.tile([P, 2, 512], F32, tag="st")
        for j, nt in enumerate(nts):
            sl = slice(nt * NT3, (nt + 1) * NT3)
            for ki in range(2):
                mm(ps[:, j, :NT3], wq_sb[:, ki, w_col0:w_col0 + P],
                   rhs_sb[:, ki, sl],
                   start=(ki == 0), stop=(ki == 1))
        lo, hi = nts[0] * NT3, (nts[-1] + 1) * NT3
        dst = dst_tile[:, lo:hi].rearrange("p (j n) -> p j n", n=NT3)
        nc.vector.tensor_copy(dst, ps[:, :len(nts), :NT3])
        dve_work(hi - lo)

    def emit_qkv_group(g):
        # q rows for group g = channels 128g..128g+127; k = 256+128g..
        qkv_mm(q_g[g], g * P, xq_sb, (0, 1))
        qkv_mm(q_g[g], g * P, xq_sb, (2,))
        for j in range(N // NT3 // 2):
            qkv_mm(k_g[g], C + g * P, xf_sb, (2 * j, 2 * j + 1))

    def emit_vt(j):
        # v^T for key chunks 2j, 2j+1 -> one fp16 evacuation
        ps = qkv_ps.tile([P, 2, 512], F32, tag="st")
        for i in range(2):
            mo = 2 * j + i
            for ki in range(2):
                mm(ps[:, i, :C], xf_sb[:, ki, mo * P:(mo + 1) * P],
                   wq_sb[:, ki, 2 * C:3 * C],
                   start=(ki == 0), stop=(ki == 1))
        nc.vector.tensor_copy(vT_c2[j][:], ps[:, :, :C])
        dve_work(2 * C)

    def emit_attention(g, pre_chunk=None):
        for qt_i, (q0, qtw) in enumerate(QT):
            av = av_ps.tile([P, 512], F32)
            sm = sm_ps.tile([P, 512], F32)
            for kc in range(KC):
                if pre_chunk is not None:
                    pre_chunk(qt_i, kc)
                # two 2-head pair tiles, pool bufs=2: the PE fills one
                # pair's banks while ACT still reads the other pair of
                # the previous chunk -> no exp->S^T serialization.
                pt = pt_pool.tile([P, 4, 512], FP16)
                for pair in range(2):
                    st = st_ps.tile([P, 2, 512], F32, tag="st")
                    for hh in range(2):
                        h = 2 * pair + hh
                        mm(st[:, hh, :qtw],
                           k_g[g][32 * h:32 * (h + 1), kc * P:(kc + 1) * P],
                           q_g[g][32 * h:32 * (h + 1), q0:q0 + qtw],
                           start=True, stop=True,
                           tile_position=(32 * h, 0))
                    emit_exp(pt[:, 2 * pair:2 * pair + 2, :qtw],
                             st[:, :, :qtw], 2 * qtw)
                for h in range(4):
                    mm(av[32 * h:32 * (h + 1), :qtw],
                       vT(kc)[:, 128 * g + 32 * h:128 * g + 32 * (h + 1)],
                       pt[:, h, :qtw],
                       start=(kc == 0), stop=(kc == KC - 1),
                       tile_position=(0, 32 * h), skip_group_check=True)
                for h in range(4):
                    mm(sm[32 * h:32 * (h + 1), :qtw],
                       ones_sb[:, :],
                       pt[:, h, :qtw],
                       start=(kc == 0), stop=(kc == KC - 1),
                       tile_position=(0, 32 * h), skip_group_check=True)
            rec = small.tile([P, 512], F32, tag="rec")
            nc.vector.reciprocal_approx_fast(out=rec[:, :qtw], in_=sm[:, :qtw])
            nc.vector.tensor_mul(av_sb[:, g, q0:q0 + qtw], av[:, :qtw],
                                 rec[:, :qtw])
            dve_work(2 * qtw)

    def emit_tail():
        # queries 1024:1152 for BOTH groups in one pass: head bank h holds
        # g0 at cols 0:128, g1 at cols 128:256. Same-row-group matmuls into
        # one bank serialize on the PE (same cells), so no concurrent
        # same-bank drains.
        q0, qtw = 1024, 128
        av = av_ps.tile([P, 512], F32)
        sm = sm_ps.tile([P, 512], F32)
        for kc in range(KC):
            pt = pt_pool.tile([P, 4, 512], FP16)
            for pair in range(2):
                st = st_ps.tile([P, 2, 512], F32, tag="st")
                for g in range(2):
                    for hh in range(2):
                        h = 2 * pair + hh
                        mm(st[:, hh, g * qtw:(g + 1) * qtw],
                           k_g[g][32 * h:32 * (h + 1), kc * P:(kc + 1) * P],
                           q_g[g][32 * h:32 * (h + 1), q0:q0 + qtw],
                           start=(g == 0), stop=(g == 1),
                           tile_position=(32 * h, 0), skip_group_check=True)
                emit_exp(pt[:, 2 * pair:2 * pair + 2, :2 * qtw],
                         st[:, :, :2 * qtw], 4 * qtw)
            for g in range(2):
                for h in range(4):
                    mm(av[32 * h:32 * (h + 1), g * qtw:(g + 1) * qtw],
                       vT(kc)[:, 128 * g + 32 * h:128 * g + 32 * (h + 1)],
                       pt[:, h, g * qtw:(g + 1) * qtw],
                       start=(kc == 0 and g == 0), stop=(kc == KC - 1 and g == 1),
                       tile_position=(0, 32 * h), skip_group_check=True)
            for g in range(2):
                for h in range(4):
                    mm(sm[32 * h:32 * (h + 1), g * qtw:(g + 1) * qtw],
                       ones_sb[:, :],
                       pt[:, h, g * qtw:(g + 1) * qtw],
                       start=(kc == 0 and g == 0), stop=(kc == KC - 1 and g == 1),
                       tile_position=(0, 32 * h), skip_group_check=True)
        rec = small.tile([P, 512], F32, tag="rec")
        nc.vector.reciprocal_approx_fast(out=rec[:, :2 * qtw], in_=sm[:, :2 * qtw])
        for g in range(2):
            nc.vector.tensor_mul(av_sb[:, g, q0:q0 + qtw],
                                 av[:, g * qtw:(g + 1) * qtw],
                                 rec[:, g * qtw:(g + 1) * qtw])
        dve_work(6 * qtw)

    y_r = y_d.rearrange("(co p) n -> p co n", p=P)

    def emit_proj(co, nts, add_on_act=False):
        ps = qkv_ps.tile([P, 2, 512], F32, tag="st")
        for j, nt in enumerate(nts):
            sl = slice(nt * NT3, (nt + 1) * NT3)
            for ki in range(2):
                mm(ps[:, j, :NT3], wp_sb[:, ki, co * P:(co + 1) * P],
                   av_sb[:, ki, sl],
                   start=(ki == 0), stop=(ki == 1))
        lo, hi = nts[0] * NT3, (nts[-1] + 1) * NT3
        dst = y_sb[:, co, lo:hi].rearrange("p (j n) -> p j n", n=NT3)
        if add_on_act:
            # drain path: ACT is idle after its last exp, so the final
            # chunks' bias-adds run concurrently on both engines
            nc.scalar.activation(dst, ps[:, :len(nts), :NT3],
                                 mybir.ActivationFunctionType.Identity,
                                 bias=bias_sb[:, co:co + 1])
        else:
            nc.vector.tensor_scalar_add(dst, ps[:, :len(nts), :NT3],
                                        bias_sb[:, co:co + 1])
            dve_work(hi - lo)
        nc.sync.dma_start(y_r[:, co, lo:hi], y_sb[:, co, lo:hi])

    # emission order shapes Tile's priorities: group A's inputs first so
    # the first exp lands as early as possible. v^T super-tiles beyond the
    # first three are emitted INSIDE attention(0)'s first query-tile sweep,
    # a few chunks ahead of their AV consumers — emitting them all up front
    # would give their DVE evacuations priority over the (critical) exp
    # stream. proj chunks for queries 0:768 only need av_sb written by the
    # full query tiles, so they are emitted before the merged tail pass and
    # fill its engine gaps; the last proj chunks follow the tail, one on
    # the qkv PSUM pool and one on the (now idle) st pool so their matmuls
    # and bias-adds overlap instead of serializing through one pool buf.
    # Output DMA is per-chunk so results ship while later chunks compute.
    emit_qkv_group(0)
    for j in range(3):
        emit_vt(j)

    # group 1's qkv super-tiles, fed into attention(0)'s SECOND query-tile
    # sweep: emitted standalone after attention(0), their DVE evacuations
    # cannot fit the near-saturated in-phase DVE gaps and stall
    # attention(1)'s first S^T (a ~3-5us ACT bubble at the transition).
    qkv1_items = [
        lambda: qkv_mm(q_g[1], P, xq_sb, (0, 1)),
        lambda: qkv_mm(k_g[1], C + P, xf_sb, (0, 1)),
        lambda: qkv_mm(k_g[1], C + P, xf_sb, (2, 3)),
        lambda: qkv_mm(q_g[1], P, xq_sb, (2,)),
        lambda: qkv_mm(k_g[1], C + P, xf_sb, (4, 5)),
    ]

    def feeder(qt_i, kc):
        if qt_i == 0 and kc % 2 == 0:
            # first sweep: v^T tiles, ~6 chunks ahead of their AV consumers
            j = kc // 2 + 3
            if j < KC // 2:
                emit_vt(j)
        elif qt_i == 1 and kc % 3 == 1:
            i = (kc - 1) // 3
            if i < len(qkv1_items):
                qkv1_items[i]()

    emit_attention(0, pre_chunk=feeder)
    emit_attention(1)
    for co in range(2):
        emit_proj(co, (0, 1))
    emit_tail()
    emit_proj(0, (2,))
    emit_proj(1, (2,), add_on_act=True)
    ctx.close()


_NC_CACHE = None


def build_nc():
    global _NC_CACHE
    if _NC_CACHE is None:
        nc = bacc.Bacc("TRN2", target_bir_lowering=False, debug=False,
                       num_devices=8)
        with tile.TileContext(nc) as tc:
            emit(tc)
        nc.compile()
        _NC_CACHE = nc
    return _NC_CACHE


def make_in_maps(x, w_qkv, w_proj, b_proj):
    import ml_dtypes
    bf16 = ml_dtypes.bfloat16
    x = np.asarray(x, np.float32).reshape(4, C, N).astype(bf16)
    wqkvT = np.ascontiguousarray(np.asarray(w_qkv, np.float32).T).astype(bf16)
    wprojT = np.ascontiguousarray(np.asarray(w_proj, np.float32).T).astype(bf16)
    bprojT = np.ascontiguousarray(np.asarray(b_proj, np.float32).reshape(2, P).T)
    in_maps = []
    for core in range(8):
        b, t = divmod(core, 2)
        in_maps.append({
            "xq": np.ascontiguousarray(x[b][:, t * NQ:(t + 1) * NQ]),
            "xf": np.ascontiguousarray(x[b]),
            "wqkvT": wqkvT,
            "wprojT": wprojT,
            "bprojT": bprojT,
        })
    return in_maps


def assemble_output(results):
    y = np.empty((4, C, N), np.float32)
    for core in range(8):
        b, t = divmod(core, 2)
        y[b][:, t * NQ:(t + 1) * NQ] = results[core]["y"]
    return y.reshape(4, C, 48, 48)


def kernel(x, w_qkv, w_proj, b_proj):
    from concourse.bass_utils import run_bass_kernel_spmd
    nc = build_nc()
    in_maps = make_in_maps(x, w_qkv, w_proj, b_proj)
    res = run_bass_kernel_spmd(nc, in_maps, core_ids=list(range(8)))
    return assemble_output(res.results)

